# revision 1
# baseline (speedup 1.0000x reference)
"""2-layer LSTM (B=256, T=512, F=49, H=512) on 8 trn2 NeuronCores.

Data-parallel over batch: core j owns batch rows [j*32, (j+1)*32), holds the
full (replicated) LSTM weights, and runs the whole recurrence locally — the
sequence dimension is strictly sequential, so there is no cross-core traffic.

Per-core kernel structure (raw bacc, fully unrolled over T):
  - orientation: h is the PE-stationary operand (lhsT [K<=128, 32]), weights
    stream as the moving operand (rhs [K, 512]) — fp32 stays stream-bound
    instead of weight-load-bound.
  - gates accumulate in two shared PSUM halves psA=[i|f], psB=[g|o]
    ([32, 1024] each); layer 2 reuses layer 1's banks once ACT consumed them.
  - the input-to-hidden term of layer 1 (x @ W_ih1^T + b_ih1 + b_hh1) rides
    the same PSUM group via a ones-row appended to x (bias-as-feature).
  - sigmoid/tanh on ScalarE, cell update on VectorE, h transposed back to
    [h, 32] chunks with PE transpose for the next step's stationary operand.
  - output projection = DVE multiply + free-axis reduce into out_sb[:, t].
"""

import sys
import types

import numpy as np

# ---------------------------------------------------------------------------
# axon NTFF profile hook (degrades silently if unavailable)
# ---------------------------------------------------------------------------


def _install_axon_hook():
    if "antenv.axon_hooks" in sys.modules:
        return
    mod = types.ModuleType("antenv.axon_hooks")
    mod._hook = None
    mod.set_axon_ntff_profile_hook = lambda h: setattr(mod, "_hook", h)
    mod.get_axon_ntff_profile_hook = lambda: mod._hook
    sys.modules["antenv.axon_hooks"] = mod
    try:
        import antenv

        antenv.axon_hooks = mod
        from trn_agent_boot.trn_boot import _ntff_profile_via_ctypes

        hook = _ntff_profile_via_ctypes("/opt/axon/libaxon_pjrt.so")
        if hook is not None:
            mod.set_axon_ntff_profile_hook(hook)
    except Exception:
        pass


_install_axon_hook()

import concourse.bacc as bacc
import concourse.mybir as mybir
from concourse.bass_utils import run_bass_kernel_spmd

DT = mybir.dt.float32
BF = mybir.dt.bfloat16
AF = mybir.ActivationFunctionType

B, T_FULL, F, H = 256, 512, 49, 512
BL = 32  # batch rows per core
FCH = 50  # features + ones row


def build(T=T_FULL):
    TCH = min(64, T)
    n_chunks = (T + TCH - 1) // TCH
    nc = bacc.Bacc("TRN2", target_bir_lowering=False)

    xw_d = nc.dram_tensor("xw", [FCH, T, BL], BF, kind="ExternalInput")
    wih1_d = nc.dram_tensor("wih1", [FCH, 2048], BF, kind="ExternalInput")
    whh1_d = nc.dram_tensor("whh1", [128, 4 * 2048], BF, kind="ExternalInput")
    wih2_d = nc.dram_tensor("wih2", [128, 4 * 2048], BF, kind="ExternalInput")
    whh2_d = nc.dram_tensor("whh2", [128, 4 * 2048], BF, kind="ExternalInput")
    b2_d = nc.dram_tensor("b2", [1, 2048], BF, kind="ExternalInput")
    woutb_d = nc.dram_tensor("woutb", [BL, 512], DT, kind="ExternalInput")
    ident_d = nc.dram_tensor("ident", [BL, BL], DT, kind="ExternalInput")
    out_d = nc.dram_tensor("out", [BL, T], DT, kind="ExternalOutput")

    xw_sb = nc.alloc_sbuf_tensor("xw_sb", [FCH, 2, TCH, BL], BF)
    wih1 = nc.alloc_sbuf_tensor("wih1_sb", [FCH, 2048], BF)
    whh1 = nc.alloc_sbuf_tensor("whh1_sb", [128, 4 * 2048], BF)
    wih2 = nc.alloc_sbuf_tensor("wih2_sb", [128, 4 * 2048], BF)
    whh2 = nc.alloc_sbuf_tensor("whh2_sb", [128, 4 * 2048], BF)
    b2 = nc.alloc_sbuf_tensor("b2_sb", [1, 2048], BF)
    woutb = nc.alloc_sbuf_tensor("woutb_sb", [BL, 512], DT)
    ident = nc.alloc_sbuf_tensor("ident_sb", [BL, BL], DT)
    ones = nc.alloc_sbuf_tensor("ones_sb", [1, BL], BF)

    hT1 = nc.alloc_sbuf_tensor("hT1", [128, 4 * BL], BF)
    hT2 = nc.alloc_sbuf_tensor("hT2", [128, 4 * BL], BF)

    def par(name, w):
        return [nc.alloc_sbuf_tensor(f"{name}_{p}", [BL, w], DT) for p in range(2)]

    c1, c2 = par("c1", 512), par("c2", 512)
    sA1, sA2 = par("sA1", 1024), par("sA2", 1024)
    tg1, tg2 = par("tg1", 512), par("tg2", 512)
    so1, so2 = par("so1", 512), par("so2", 512)
    thc1, thc2 = par("thc1", 512), par("thc2", 512)
    tm1 = nc.alloc_sbuf_tensor("tm1", [BL, 512], DT)
    tm2 = nc.alloc_sbuf_tensor("tm2", [BL, 512], DT)
    ta1 = nc.alloc_sbuf_tensor("ta1", [BL, 512], DT)
    ta2 = nc.alloc_sbuf_tensor("ta2", [BL, 512], DT)
    h1, h2 = par("h1", 512), par("h2", 512)
    pm = nc.alloc_sbuf_tensor("pm", [BL, 512], DT)
    out_sb = nc.alloc_sbuf_tensor("out_sb", [BL, T], DT)

    psA = nc.alloc_psum_tensor("psA", [BL, 1024], DT)
    psB = nc.alloc_psum_tensor("psB", [BL, 1024], DT)
    ptr = nc.alloc_psum_tensor("ptr", [128, 2 * 4 * BL], DT)

    S = lambda n: nc.alloc_semaphore(n)
    sem_w, sem_x, sem_ones = S("sem_w"), S("sem_x"), S("sem_ones")
    pe1a, pe1b, pe2a, pe2b = S("pe1a"), S("pe1b"), S("pe2a"), S("pe2b")
    a1a, a1b, a2a, a2b = S("a1a"), S("a1b"), S("a2a"), S("a2b")
    at1, at2 = S("at1"), S("at2")
    sc1, sc2 = S("sc1"), S("sc2")
    he1, he2 = S("he1"), S("he2")
    tr1s, tr2s = S("tr1s"), S("tr2s")
    ho1, ho2 = S("ho1"), S("ho2")
    prj = S("prj")

    with nc.Block() as block:

        @block.sync
        def _(sync):
            for dst, src in [
                (wih1, wih1_d), (whh1, whh1_d), (wih2, wih2_d), (whh2, whh2_d),
                (b2, b2_d), (woutb, woutb_d), (ident, ident_d),
            ]:
                sync.dma_start(out=dst[:], in_=src[:]).then_inc(sem_w, 16)
            for c in range(n_chunks):
                if c >= 2:
                    sync.wait_ge(pe1b, TCH * (c - 1))
                inst = sync.dma_start(
                    out=xw_sb[:, c % 2], in_=xw_d[:, c * TCH : (c + 1) * TCH, :]
                )
                if c >= 1:
                    inst._wait_ge(sem_x, 16 * c)  # previous chunk landed before issuing
                inst.then_inc(sem_x, 16)
            sync.wait_ge(prj, T)
            sync.dma_start(out=out_d[:], in_=out_sb[:]).then_inc(sem_x, 16)
            sync.wait_ge(sem_x, 16 * (n_chunks + 1))

        @block.tensor
        def _(pe):
            pe.wait_ge(sem_w, 7 * 16)
            pe.wait_ge(sem_ones, 1)

            def mm_region(ps, n, first, rhs_sb, co, k, start, stop):
                lhs = first if k is None else (rhs_sb, k)
                return None

            for t in range(T):
                pi = t % 2
                if t % TCH == 0:
                    pe.wait_ge(sem_x, 16 * (t // TCH + 1))
                xt = xw_sb[:, (t // TCH) % 2, t % TCH, :]

                def gemm_l1(ps, off):
                    for n in range(2):
                        reg = ps[:, n * 512 : (n + 1) * 512]
                        co = off + n * 512
                        mm = pe.matmul(reg, xt, wih1[:, co : co + 512],
                                       start=True, stop=(t == 0))
                        if t >= 1:
                            for k in range(4):
                                mm = pe.matmul(
                                    reg,
                                    hT1[:, k * BL : (k + 1) * BL],
                                    whh1[:, k * 2048 + co : k * 2048 + co + 512],
                                    start=False, stop=(k == 3),
                                )
                    return mm

                def gemm_l2_p1(ps, off):  # bias + hh2 (no stop)
                    for n in range(2):
                        reg = ps[:, n * 512 : (n + 1) * 512]
                        co = off + n * 512
                        mm = pe.matmul(reg, ones[:], b2[:, co : co + 512],
                                       start=True, stop=False)
                        if t >= 1:
                            for k in range(4):
                                mm = pe.matmul(
                                    reg,
                                    hT2[:, k * BL : (k + 1) * BL],
                                    whh2[:, k * 2048 + co : k * 2048 + co + 512],
                                    start=False, stop=False,
                                )
                    return mm

                def gemm_l2_p2(ps, off):  # ih2 (stop)
                    for n in range(2):
                        reg = ps[:, n * 512 : (n + 1) * 512]
                        co = off + n * 512
                        for k in range(4):
                            mm = pe.matmul(
                                reg,
                                hT1[:, k * BL : (k + 1) * BL],
                                wih2[:, k * 2048 + co : k * 2048 + co + 512],
                                start=False, stop=(k == 3),
                            )
                    return mm

                # ---- L1 gemm (uses hT1 = h1[t-1])
                if t >= 1:
                    pe.wait_ge(a2a, t)   # psA free (ACT read L2A of t-1)
                    pe.wait_ge(ho1, t)   # hT1 holds h1[t-1]
                gemm_l1(psA, 0).then_inc(pe1a, 1)
                if t >= 1:
                    pe.wait_ge(a2b, t)
                gemm_l1(psB, 1024).then_inc(pe1b, 1)
                # ---- transpose h2[t-1] (pipelined from previous step)
                if t >= 1:
                    pe.wait_ge(he2, t)
                    if t >= 2:
                        pe.wait_ge(ho2, t - 1)  # ptr cols 128:256 WAR
                    po_prev = (t - 1) % 2
                    for k in range(4):
                        mm = pe.transpose(
                            ptr[:, 128 + k * BL : 128 + (k + 1) * BL],
                            h2[po_prev][:, k * 128 : (k + 1) * 128],
                            ident[:],
                        )
                    mm.then_inc(tr2s, 1)
                # ---- L2 bias + hh2 (uses hT2 = h2[t-1])
                pe.wait_ge(a1a, t + 1)   # psA free (ACT read L1A of t)
                if t >= 1:
                    pe.wait_ge(ho2, t)   # hT2 holds h2[t-1]
                gemm_l2_p1(psA, 0)
                pe.wait_ge(a1b, t + 1)
                gemm_l2_p1(psB, 1024)
                # ---- transpose h1[t]
                pe.wait_ge(he1, t + 1)
                if t >= 1:
                    pe.wait_ge(ho1, t)   # ptr cols 0:128 WAR vs copy of t-1
                for k in range(4):
                    mm = pe.transpose(
                        ptr[:, k * BL : (k + 1) * BL],
                        h1[pi][:, k * 128 : (k + 1) * 128],
                        ident[:],
                    )
                mm.then_inc(tr1s, 1)
                # ---- L2 ih2 (uses hT1 = h1[t])
                pe.wait_ge(ho1, t + 1)
                gemm_l2_p2(psA, 0).then_inc(pe2a, 1)
                gemm_l2_p2(psB, 1024).then_inc(pe2b, 1)
            # final h2 transpose so DVE's last copy unblocks
            pe.wait_ge(he2, T)
            pe.wait_ge(ho2, T - 1)
            for k in range(4):
                mm = pe.transpose(
                    ptr[:, 128 + k * BL : 128 + (k + 1) * BL],
                    h2[(T - 1) % 2][:, k * 128 : (k + 1) * 128],
                    ident[:],
                )
            mm.then_inc(tr2s, 1)

        @block.scalar
        def _(act):
            for t in range(T):
                pi = t % 2
                for (pa, pb, sA, tg, so, thc, cc, ra, rb, att, scs) in [
                    (pe1a, pe1b, sA1, tg1, so1, thc1, c1, a1a, a1b, at1, sc1),
                    (pe2a, pe2b, sA2, tg2, so2, thc2, c2, a2a, a2b, at2, sc2),
                ]:
                    act.wait_ge(pa, t + 1)
                    act.activation(sA[pi][:], psA[:], AF.Sigmoid).then_inc(ra, 1)
                    act.wait_ge(pb, t + 1)
                    act.activation(tg[pi][:], psB[:, 0:512], AF.Tanh)
                    act.activation(so[pi][:], psB[:, 512:1024], AF.Sigmoid).then_inc(rb, 1)
                    act.wait_ge(scs, t + 1)
                    act.activation(thc[pi][:], cc[pi][:], AF.Tanh).then_inc(att, 1)

        @block.vector
        def _(dve):
            for t in range(T):
                pi = t % 2
                po = 1 - pi
                for li, (sA, tg, so, thc, cc, tm, ta, hh, rb, att, scs, hes, trs, hos, hT) in enumerate([
                    (sA1, tg1, so1, thc1, c1, tm1, ta1, h1, a1b, at1, sc1, he1, tr1s, ho1, hT1),
                    (sA2, tg2, so2, thc2, c2, tm2, ta2, h2, a2b, at2, sc2, he2, tr2s, ho2, hT2),
                ]):
                    dve.wait_ge(rb, t + 1)
                    if t >= 1:
                        dve.drain()  # cross-step same-engine RAW (c tiles)
                    if t == 0:
                        dve.tensor_mul(cc[pi][:], sA[pi][:, 0:512], tg[pi][:]).then_inc(scs, 1)
                    else:
                        dve.tensor_mul(tm[:], sA[pi][:, 0:512], tg[pi][:])
                        dve.tensor_mul(ta[:], sA[pi][:, 512:1024], cc[po][:])
                        dve.drain()
                        dve.tensor_add(cc[pi][:], ta[:], tm[:]).then_inc(scs, 1)
                    dve.wait_ge(att, t + 1)
                    if t >= 2:
                        dve.wait_ge(trs, t - 1)  # h buf WAR vs PE transpose reads
                    dve.tensor_mul(hh[pi][:], so[pi][:], thc[pi][:]).then_inc(hes, 1)
                    off = li * 128
                    dve.wait_ge(trs, t + 1)
                    dve.tensor_copy(hT[:], ptr[:, off : off + 128]).then_inc(hos, 1)
                dve.drain()
                dve.tensor_mul(pm[:], h2[pi][:], woutb[:])
                dve.drain()
                dve.reduce_sum(
                    out_sb[:, t : t + 1], pm[:], axis=mybir.AxisListType.X
                ).then_inc(prj, 1)

        @block.gpsimd
        def _(gp):
            gp.memset(ones[:], 1.0).then_inc(sem_ones, 1)

    nc.compile()
    return nc


def prepack(inputs, core):
    x = np.asarray(inputs["x"], dtype=np.float32)
    T = x.shape[1]
    w_ih1 = np.asarray(inputs["w_ih1"], dtype=np.float32)
    w_hh1 = np.asarray(inputs["w_hh1"], dtype=np.float32)
    b1 = np.asarray(inputs["b_ih1"], dtype=np.float32) + np.asarray(
        inputs["b_hh1"], dtype=np.float32
    )
    w_ih2 = np.asarray(inputs["w_ih2"], dtype=np.float32)
    w_hh2 = np.asarray(inputs["w_hh2"], dtype=np.float32)
    b2v = np.asarray(inputs["b_ih2"], dtype=np.float32) + np.asarray(
        inputs["b_hh2"], dtype=np.float32
    )
    w_out = np.asarray(inputs["w_out"], dtype=np.float32)

    xs = x[core * BL : (core + 1) * BL]  # [32, T, 49]
    xw = np.empty((FCH, T, BL), np.float32)
    xw[:F] = np.transpose(xs, (2, 1, 0))
    xw[F] = 1.0

    wih1 = np.empty((FCH, 2048), np.float32)
    wih1[:F] = w_ih1.T
    wih1[F] = b1

    def hh_pack(w):  # [2048, 512] -> [128, 4*2048]: [r, k*2048+c] = w[c, k*128+r]
        out = np.empty((128, 4 * 2048), np.float32)
        for k in range(4):
            out[:, k * 2048 : (k + 1) * 2048] = np.ascontiguousarray(
                w[:, k * 128 : (k + 1) * 128].T
            )
        return out

    import ml_dtypes

    bf16 = ml_dtypes.bfloat16
    return {
        "xw": np.ascontiguousarray(xw).astype(bf16),
        "wih1": wih1.astype(bf16),
        "whh1": hh_pack(w_hh1).astype(bf16),
        "wih2": hh_pack(w_ih2).astype(bf16),
        "whh2": hh_pack(w_hh2).astype(bf16),
        "b2": np.ascontiguousarray(b2v[None, :]).astype(bf16),
        "woutb": np.tile(w_out[0][None, :], (BL, 1)).astype(np.float32),
        "ident": np.eye(BL, dtype=np.float32),
    }


_NC_CACHE = {}


def _get_nc(T):
    if T not in _NC_CACHE:
        _NC_CACHE[T] = build(T)
    return _NC_CACHE[T]


def kernel(**inputs):
    """Full-input entry: shard over 8 cores, run, gather. Returns [B, T] fp32."""
    x = np.asarray(inputs["x"])
    T = x.shape[1]
    nc = _get_nc(T)
    in_maps = [prepack(inputs, j) for j in range(8)]
    res = run_bass_kernel_spmd(nc, in_maps, core_ids=list(range(8)))
    out = np.empty((B, T), np.float32)
    for j in range(8):
        out[j * BL : (j + 1) * BL] = res.results[j]["out"][:, :T]
    out += np.asarray(inputs["b_out"], dtype=np.float32)[0]
    return out

